# revision 1
# baseline (speedup 1.0000x reference)
"""Bahdanau-style additive attention on 8 TRN2 NeuronCores.

Reference computation (B=32, S=2048, H=1024):
    query  = hidden @ Wq.T                      # (B, H)
    keys   = enc @ Wk.T                         # (B, S, H)
    energy = tanh(query[:, None, :] + keys)     # (B, S, H)
    attn   = energy @ v                         # (B, S)
    out    = softmax(mask(attn, lengths))       # (B, S)

Key observation: seq positions >= lengths[b] are masked to -inf and
contribute exactly 0 to the softmax, so their keys/energy never need to
be computed.  The host packs only the *live* 512-wide seq tiles
("slots") and bin-packs whole batches across the 8 cores to equalize
slot counts (NT slots per core, vs 16 for the dense layout).  Every
core runs an identical program over its NT slots; all per-slot
metadata (query column, mask row, softmax assignment one-hots) is
data, not program, so the SPMD program stays uniform while cores
process different (batch, seq-tile) sets.

Softmax runs without a per-batch running max: a fixed logit shift
(baked into the host-built mask bias) keeps exp() inside fp32 range --
mathematically exact for softmax -- and removes the per-tile max-reduce
chains of the dense version.  Per-batch denominators are formed on the
PE with tiny one-hot assignment matmuls and broadcast back the same
way, since per-core batch->slot structure is data.

Layout: slots are grouped 4 to a PSUM bank; slot t lives at partition
32*(t%4) of group g=t//4 (engine APs must start 32-aligned).  The
v-contraction uses an expanded stationary [128,128] with v in column
32*(t%4), so one accumulation group per PSUM bank covers 4 slots and
every element is written (no unwritten-PSUM garbage).

Per-slot dataflow (all matmuls fp32r - ~120ns per [128x128]@[128x512]):
    - enc slot tile arrives pre-transposed [H, 512]; keys tile
      [128f, 512s] accumulates 8 h-chunk matmuls in PSUM.
    - ACT applies tanh with the slot's per-partition query bias.
    - PE contracts energy with v_ext into the slot's group PSUM row
      (deferred ~1 PE group to hide the tanh latency).
    - per finished group: DVE adds mask/shift, ACT exps with per-row
      accumulate, PE folds the row-sums into per-batch denominators.
    - iteration tail: reciprocal, broadcast back to slot rows via a
      second one-hot matmul, scale, DMA out.  Host scatters slot rows
      back to (batch, seq).
"""

import sys

if "/opt/trn_rl_repo" not in sys.path:
    sys.path.insert(0, "/opt/trn_rl_repo")

import ml_dtypes
import numpy as np

B, S, H = 32, 2048, 1024
NCORES = 8
FT = 128           # partition tile (feature / h chunk)
HC = H // FT       # h chunks
ST = 512           # seq tile
NST = S // ST
NT = 10            # packed slots per core (max over cores after balancing)
NG = 3             # slot groups of 4 (ceil(NT/4))
NB = 8             # max distinct batches per core
SHIFT = 30.0       # constant logit shift baked into the mask bias

_CACHE = {}


def _build(variant="full", loop_r=1, nt=NT):
    import concourse.bass as bass  # noqa: F401
    import concourse.tile as tile
    from concourse import bacc, mybir

    ng = (nt + 3) // 4
    ntp = nt + (nt % 2)  # fp32r matmuls need an even moving/dest free dim
    f32 = mybir.dt.float32
    f32r = mybir.dt.float32r
    bf16 = mybir.dt.bfloat16
    Tanh = mybir.ActivationFunctionType.Tanh
    Exp = mybir.ActivationFunctionType.Exp

    nc = bacc.Bacc("TRN2", target_bir_lowering=False, debug=False,
                   num_devices=NCORES)

    encP = nc.dram_tensor("encP", [nt, H, ST], bf16, kind="ExternalInput").ap()
    hselT = nc.dram_tensor("hselT", [H, ntp], f32r, kind="ExternalInput").ap()
    wkT = nc.dram_tensor("wkT", [H, H], bf16, kind="ExternalInput").ap()
    wqT = nc.dram_tensor("wqT", [H, H], f32r, kind="ExternalInput").ap()
    vp = nc.dram_tensor("vp", [FT, HC], f32r, kind="ExternalInput").ap()
    maskP = nc.dram_tensor("maskP", [FT, ng, ST], f32,
                           kind="ExternalInput").ap()
    asn = nc.dram_tensor("asn", [ng, FT, NB], bf16, kind="ExternalInput").ap()
    asn2 = nc.dram_tensor("asn2", [NB, ng, FT], bf16,
                          kind="ExternalInput").ap()
    dbias = nc.dram_tensor("dbias", [NB, 1], f32, kind="ExternalInput").ap()
    out = nc.dram_tensor("out", [ng, 4, ST], f32, kind="ExternalOutput").ap()
    if variant == "debug":
        dbg_q = nc.dram_tensor("dbg_q", [FT, HC, ntp], f32,
                               kind="ExternalOutput").ap()
        dbg_en = nc.dram_tensor("dbg_en", [FT, ST], f32r,
                                kind="ExternalOutput").ap()
        dbg_att = nc.dram_tensor("dbg_att", [FT, ng, ST], f32,
                                 kind="ExternalOutput").ap()
        dbg_rs = nc.dram_tensor("dbg_rs", [FT, ng], f32,
                                kind="ExternalOutput").ap()
        dbg = {"q": dbg_q, "en": dbg_en, "att": dbg_att, "rs": dbg_rs}
    else:
        dbg = None

    with tile.TileContext(nc) as tc:
        with (
            tc.tile_pool(name="singles", bufs=1) as singles,
            tc.tile_pool(name="encp", bufs=3) as encp,
            tc.tile_pool(name="energy", bufs=4) as ep,
            tc.tile_pool(name="kpsum", bufs=4, space="PSUM") as kps,
            tc.tile_pool(name="apsum", bufs=2, space="PSUM") as aps,
            tc.tile_pool(name="spsum", bufs=1, space="PSUM") as sps,
            tc.tile_pool(name="stats", bufs=1) as stats,
        ):
            wk_sb = singles.tile([FT, HC, H], bf16)
            wq_sb = singles.tile([FT, HC, H], f32r)
            hsel_sb = singles.tile([FT, HC, ntp], f32r)
            v_sb = singles.tile([FT, HC], f32r)
            mask_sb = singles.tile([FT, ng, ST], f32)
            qsel_sb = singles.tile([FT, HC, ntp], f32)
            attn_sb = singles.tile([FT, ng, ST], f32)
            asn_sb = singles.tile([FT, ng, NB], bf16)
            asn2_sb = singles.tile([NB, ng, FT], bf16)
            dbias_sb = singles.tile([NB, 1], f32)

            for hc in range(HC):
                nc.sync.dma_start(out=wk_sb[:, hc, :],
                                  in_=wkT[hc * FT:(hc + 1) * FT, :])
                nc.sync.dma_start(out=wq_sb[:, hc, :],
                                  in_=wqT[hc * FT:(hc + 1) * FT, :])
                nc.sync.dma_start(out=hsel_sb[:, hc, :],
                                  in_=hselT[hc * FT:(hc + 1) * FT, :])
            nc.sync.dma_start(out=v_sb[:], in_=vp[:])
            nc.sync.dma_start(out=mask_sb[:], in_=maskP[:])
            for g in range(ng):
                nc.sync.dma_start(out=asn_sb[:, g, :], in_=asn[g])
            nc.sync.dma_start(out=asn2_sb[:], in_=asn2[:])
            nc.sync.dma_start(out=dbias_sb[:], in_=dbias[:])

            # q projection for every slot: qsel[f, t] = sum_h WqT[h, f] *
            # hsel[h, t].  All 8 fc regions share one PSUM tile (start only
            # on the global first matmul, per-element has_written covers the
            # rest; nothing reads until the global stop).
            qp = aps.tile([FT, HC, ntp], f32, tag="gp")
            for hc in range(HC):
                for fc in range(HC):
                    nc.tensor.matmul(
                        qp[:, fc, :],
                        lhsT=wq_sb[:, hc, fc * FT:(fc + 1) * FT],
                        rhs=hsel_sb[:, hc, :],
                        start=(hc == 0 and fc == 0),
                        stop=(hc == HC - 1 and fc == HC - 1),
                        skip_group_check=True)
            nc.vector.tensor_copy(out=qsel_sb[:], in_=qp[:])
            # non-slot rows of attn_sb are never written by the slot adds;
            # park them at -1e30 so the group exp yields exactly 0 there
            nc.vector.memset(attn_sb[:], -30.0)

            def load_et(t):
                et = encp.tile([FT, HC, ST], bf16, tag="et", name="et")
                for hc in range(HC):
                    nc.sync.dma_start(
                        out=et[:, hc, :],
                        in_=encP[t, hc * FT:(hc + 1) * FT, :])
                return et

            if dbg is not None:
                nc.sync.dma_start(out=dbg["q"], in_=qsel_sb[:])
            args = (nc, tc, mybir, f32, f32r, bf16, Tanh, Exp, variant,
                    nt, ng, load_et, out, ep, kps, aps, sps, stats,
                    dbg,
                    wk_sb, v_sb, mask_sb, qsel_sb, attn_sb,
                    asn_sb, asn2_sb, dbias_sb)
            if loop_r > 1:
                with tc.For_i(0, loop_r, 1):
                    _body(*args, pipelined=True)
            else:
                _body(*args, pipelined=False)

    nc.compile()
    return nc


def _body(nc, tc, mybir, f32, f32r, bf16, Tanh, Exp, variant, nt, ng,
          load_et, out, ep, kps, aps, sps, stats, dbg,
          wk_sb, v_sb, mask_sb, qsel_sb, attn_sb,
          asn_sb, asn2_sb, dbias_sb, pipelined=False):
    """Emit the per-iteration slot loop + softmax tail.

    The v-matvec for a finished energy tile is emitted one PE group late
    (after matmul hc==4 of the next keys group) so the tanh latency never
    stalls the in-order PE queue.  Groups 0..ng-2 finish their softmax
    prefix (mask add, exp, denominator fold) while later slots are still
    streaming keys, so only group ng-1's short chain sits at the
    iteration boundary.
    """
    HC_ = HC
    pending_v = []  # (ap_tile, energy_ap, fc, t)
    state = {"ap": None, "dn": None, "rowsum": None, "rowsum_bf": None}

    def slot_tail(t, pap):
        # scatter the finished [1,512] attn row into its 32-aligned slot
        # row, folding in the mask/shift bias
        g, p = t // 4, t % 4
        nc.vector.tensor_add(
            out=attn_sb[32 * p:32 * p + 1, g, :], in0=pap[:],
            in1=mask_sb[32 * p:32 * p + 1, g, :])
        if t == min(4 * g + 3, nt - 1) and not (pipelined and
                                                g == ng - 1):
            group_tail(g)

    def group_tail(g):
        if dbg is not None:
            nc.sync.dma_start(out=dbg["att"][:, g, :], in_=attn_sb[:, g, :])
        if state["rowsum"] is None:
            state["rowsum"] = stats.tile([FT, ng], f32, tag="rs", name="rowsum")
            state["rowsum_bf"] = stats.tile([FT, ng], bf16, tag="rsb",
                                            name="rowsum_bf")
            state["dn"] = sps.tile([NB, 1], f32, tag="dn", name="dn")
        nc.scalar.activation(
            out=attn_sb[:, g, :], in_=attn_sb[:, g, :], func=Exp,
            bias=0.0, scale=1.0,
            accum_out=state["rowsum"][:, g:g + 1])
        with nc.allow_low_precision(reason="bf16 one-hot matmul operand"):
            nc.vector.tensor_copy(out=state["rowsum_bf"][:, g:g + 1],
                                  in_=state["rowsum"][:, g:g + 1])
        nc.tensor.matmul(
            state["dn"][:], lhsT=asn_sb[:, g, :],
            rhs=state["rowsum_bf"][:, g:g + 1],
            start=(g == 0), stop=(g == ng - 1))

    def iter_tail():
        denom = stats.tile([NB, 1], f32, tag="den", name="denom")
        nc.vector.tensor_add(out=denom[:], in0=state["dn"][:],
                             in1=dbias_sb[:])
        recip = stats.tile([NB, 1], bf16, tag="rc", name="recip")
        with nc.allow_low_precision(reason="bf16 one-hot matmul operand"):
            nc.vector.reciprocal(out=recip[:], in_=denom[:])
        rs_ps = sps.tile([FT, ng], f32, tag="rsp", name="rs_ps")
        for g in range(ng):
            nc.tensor.matmul(rs_ps[:, g:g + 1], lhsT=asn2_sb[:, g, :],
                             rhs=recip[:], start=True, stop=True,
                             skip_group_check=True)
        rslot = stats.tile([FT, ng], f32, tag="rsl", name="rslot")
        nc.vector.tensor_copy(out=rslot[:], in_=rs_ps[:])
        for g in range(ng):
            nc.vector.tensor_scalar_mul(attn_sb[:, g, :], attn_sb[:, g, :],
                                        rslot[:, g:g + 1])
            nc.sync.dma_start(out=out[g], in_=attn_sb[0:FT:32, g, :])

    def flush(n):
        for _ in range(min(n, len(pending_v))):
            pap, pen, pfc, pt = pending_v.pop(0)
            nc.tensor.matmul(
                pap[:], lhsT=v_sb[:, pfc:pfc + 1], rhs=pen,
                start=(pfc == 0), stop=(pfc == HC_ - 1))
            if pfc == HC_ - 1:
                slot_tail(pt, pap)

    def keys_group(et, fc):
        kp = kps.tile([FT, ST], f32, tag="kp")
        for hc in range(HC_):
            nc.tensor.matmul(
                kp[:],
                lhsT=wk_sb[:, hc, fc * FT:(fc + 1) * FT],
                rhs=et[:, hc, :],
                start=(hc == 0), stop=(hc == HC_ - 1))
            if hc in (4, 7):
                flush(2)
        return kp

    if pipelined:
        # previous iteration's last-group exp/denominator fold + its full
        # softmax tail run here, overlapped with this iteration's early
        # keys matmuls (their PE deps resolve ~20us before the PE arrives)
        group_tail(ng - 1)
        iter_tail()
    for t in range(nt):
        et = load_et(t)
        state["ap"] = aps.tile([1, ST], f32, tag="gp", name="ap_")
        ap_ = state["ap"]
        for fc in range(HC_):
            kp = keys_group(et, fc)
            en = ep.tile([FT, ST], f32r, tag="en")
            nc.scalar.activation(
                out=en[:], in_=kp[:], func=Tanh,
                bias=qsel_sb[:, fc, t:t + 1], scale=1.0)
            if dbg is not None and t == 0 and fc == 0:
                nc.sync.dma_start(out=dbg["en"], in_=en[:])
            pending_v.append((ap_, en[:], fc, t))
    flush(len(pending_v))
    if pipelined:
        return

    # ---- single-shot tail: denominators -> slot scales -> output ----
    if dbg is not None:
        nc.sync.dma_start(out=dbg["rs"], in_=state["rowsum"][:])
    iter_tail()


def _plan(lengths):
    """Bin-pack whole batches across cores to equalize live-slot counts.

    Returns (assignment, nt_needed): assignment[c] = list of
    (global_batch, n_tiles); deterministic in `lengths`.
    """
    ntiles = [max(1, int(-(-int(l) // ST))) for l in lengths]
    order = sorted(range(B), key=lambda b: (-ntiles[b], b))
    sums = [0] * NCORES
    counts = [0] * NCORES
    assignment = [[] for _ in range(NCORES)]
    for b in order:
        cand = sorted(range(NCORES), key=lambda c: (sums[c], counts[c], c))
        placed = False
        for c in cand:
            if counts[c] < NB:
                assignment[c].append((b, ntiles[b]))
                sums[c] += ntiles[b]
                counts[c] += 1
                placed = True
                break
        assert placed
    return assignment, max(sums)


def _prepare_in_maps(hidden, encoder_outputs, lengths, Wq, Wk, v):
    hidden = np.asarray(hidden, dtype=np.float32)
    enc = np.asarray(encoder_outputs, dtype=np.float32)
    lengths = np.asarray(lengths).astype(np.int64)
    Wq = np.asarray(Wq, dtype=np.float32)
    Wk = np.asarray(Wk, dtype=np.float32)
    v = np.asarray(v, dtype=np.float32)

    assignment, nt_needed = _plan(lengths)
    nt = max(NT, nt_needed)
    ng = (nt + 3) // 4
    ntp = nt + (nt % 2)
    _CACHE["nt"] = nt
    placement = []  # (core, slot, global_batch, seq_tile)

    WkT = np.ascontiguousarray(Wk.T).astype(ml_dtypes.bfloat16)  # (H, H)
    WqT = np.ascontiguousarray(Wq.T)                             # (H, H)
    encT = enc.transpose(0, 2, 1)                                # (B, H, S)
    vp = np.ascontiguousarray(v.reshape(HC, FT).T)               # (128, 8)

    in_maps = []
    for c in range(NCORES):
        encP = np.zeros((nt, H, ST), dtype=ml_dtypes.bfloat16)
        hselT = np.zeros((H, ntp), dtype=np.float32)
        maskP = np.full((FT, ng, ST), np.float32(-1e30), dtype=np.float32)
        asn = np.zeros((ng, FT, NB), dtype=np.float32)
        dbias = np.ones((NB, 1), dtype=np.float32)
        t = 0
        for j, (b, ntb) in enumerate(assignment[c]):
            L = int(lengths[b])
            dbias[j, 0] = 0.0
            for st in range(ntb):
                g, p = t // 4, t % 4
                encP[t] = encT[b, :, st * ST:(st + 1) * ST]
                hselT[:, t] = hidden[b]
                nvalid = min(ST, L - st * ST)
                maskP[32 * p, g, :nvalid] = -SHIFT
                asn[g, 32 * p, j] = 1.0
                placement.append((c, t, b, st))
                t += 1
        asn2 = np.ascontiguousarray(
            asn.transpose(2, 0, 1)).astype(ml_dtypes.bfloat16)   # (NB, ng, FT)
        in_maps.append({
            "encP": encP,
            "hselT": np.ascontiguousarray(hselT),
            "wkT": WkT,
            "wqT": WqT,
            "vp": vp,
            "maskP": maskP,
            "asn": asn.astype(ml_dtypes.bfloat16),
            "asn2": asn2,
            "dbias": dbias,
        })
    _CACHE["placement"] = placement
    return in_maps


def _get_nc():
    nt = _CACHE.get("nt", NT)
    key = ("nc", nt)
    if key not in _CACHE:
        _CACHE[key] = _build(nt=nt)
    return _CACHE[key]


def _run(in_maps, trace=False, **kw):
    from concourse.bass_utils import run_bass_kernel_spmd
    nc = _get_nc()
    res = run_bass_kernel_spmd(nc, in_maps, core_ids=list(range(NCORES)),
                               trace=trace, **kw)
    outs = np.zeros((B, S), dtype=np.float32)
    for (c, t, b, st) in _CACHE["placement"]:
        outs[b, st * ST:(st + 1) * ST] = res.results[c]["out"][t // 4, t % 4]
    return outs, res


def kernel(hidden, encoder_outputs, lengths, Wq, Wk, v):
    in_maps = _prepare_in_maps(hidden, encoder_outputs, lengths, Wq, Wk, v)
    outs, _ = _run(in_maps, trace=False)
    return outs



# revision 5
# speedup vs baseline: 1.1639x; 1.1639x over previous
"""Bahdanau-style additive attention on 8 TRN2 NeuronCores.

Reference computation (B=32, S=2048, H=1024):
    query  = hidden @ Wq.T                      # (B, H)
    keys   = enc @ Wk.T                         # (B, S, H)
    energy = tanh(query[:, None, :] + keys)     # (B, S, H)
    attn   = energy @ v                         # (B, S)
    out    = softmax(mask(attn, lengths))       # (B, S)

Seq positions >= lengths[b] are masked to -inf and contribute exactly 0
to the softmax, so their keys/energy never need to be computed.

v2 layout: the host packs the *live* seq range of every batch into
128-col tiles (TS=128) and bin-packs whole batches across the 8 cores
by tile count.  Each core's tile stream is processed in 512-col matmul
chunks (CW=512, 4 tiles each; the ragged tail is a 128-col chunk) --
the matmul moving dim stays wide (LDWEIGHTS amortized) while the
padding quantum is 128, cutting streamed columns ~18% vs 512-slot
packing.  A chunk may mix tiles of different batches: the only
batch-dependent stages are the query bias (applied per 128-col subtile
by 4 ACT tanh ops per chunk) and the softmax bookkeeping (per-subtile
exp row-sums folded per-batch with one-hot matmuls).  All per-tile
metadata (query column, mask bias, one-hots) is data, so the SPMD
program is identical on all cores.

Softmax uses a fixed logit shift (baked into the host mask bias)
instead of a running max -- exact for softmax, no max-reduce chains.
Per-batch denominators are formed on the PE with tiny one-hot
assignment matmuls and broadcast back the same way.

Per-chunk dataflow (keys matmuls bf16, ~213ns per [128x128]@[128x512]):
    - enc chunk arrives pre-packed [128, HC, cw] (one DMA); keys tile
      [128f, cw] accumulates 8 h-chunk matmuls in PSUM.
    - ACT applies tanh per 128-col subtile with that tile's query bias.
    - PE contracts energy with v into the chunk's attn row (deferred
      ~1 PE group to hide the tanh latency).
    - per finished group of 4 chunks: DVE adds mask/shift, ACT exps per
      subtile with per-row accumulate, PE folds the (row, subtile) sums
      into per-batch denominators.
    - iteration tail: reciprocal, broadcast back to (row, subtile)
      scale factors via one-hot matmuls, scale, DMA out.  Host scatters
      tile rows back to (batch, seq).
"""

import sys

if "/opt/trn_rl_repo" not in sys.path:
    sys.path.insert(0, "/opt/trn_rl_repo")

import ml_dtypes
import numpy as np

B, S, H = 32, 2048, 1024
NCORES = 8
FT = 128           # partition tile (feature / h chunk)
HC = H // FT       # h chunks
TS = 128           # seq packing tile
CW = 512           # matmul chunk width
TPC = CW // TS     # tiles per full chunk (4)
NB = 8             # max distinct batches per core
SHIFT = 30.0       # constant logit shift baked into the mask bias

_CACHE = {}


def _chunks_of(maxtiles):
    """Chunk widths for a core holding `maxtiles` tile slots."""
    n512, rem = divmod(maxtiles, TPC)
    cs = [CW] * n512
    if rem:
        cs.append(rem * TS)
    return cs


def _build(loop_r=1, maxtiles=33):
    import concourse.bass as bass  # noqa: F401
    import concourse.tile as tile
    from concourse import bacc, mybir

    cs = _chunks_of(maxtiles)
    nch = len(cs)
    ng = (nch + 3) // 4
    nsub = ng * TPC               # (group, subtile) slots
    ntp = maxtiles + (maxtiles % 2)  # fp32r moving/dest free dim must be even
    tot = sum(cs)
    f32 = mybir.dt.float32
    f32r = mybir.dt.float32r
    bf16 = mybir.dt.bfloat16
    Tanh = mybir.ActivationFunctionType.Tanh
    Exp = mybir.ActivationFunctionType.Exp

    nc = bacc.Bacc("TRN2", target_bir_lowering=False, debug=False,
                   num_devices=NCORES)

    encP = nc.dram_tensor("encP", [FT, HC, tot], bf16,
                          kind="ExternalInput").ap()
    hselT = nc.dram_tensor("hselT", [H, ntp], f32r, kind="ExternalInput").ap()
    wkT = nc.dram_tensor("wkT", [H, H], bf16, kind="ExternalInput").ap()
    wqT = nc.dram_tensor("wqT", [H, H], f32r, kind="ExternalInput").ap()
    vp = nc.dram_tensor("vp", [FT, HC], f32r, kind="ExternalInput").ap()
    maskP = nc.dram_tensor("maskP", [FT, ng, CW], f32,
                           kind="ExternalInput").ap()
    asn = nc.dram_tensor("asn", [FT, nsub, NB], bf16,
                         kind="ExternalInput").ap()
    asn2 = nc.dram_tensor("asn2", [NB, nsub, FT], bf16,
                          kind="ExternalInput").ap()
    dbias = nc.dram_tensor("dbias", [NB, 1], f32, kind="ExternalInput").ap()
    out = nc.dram_tensor("out", [ng, 4, CW], f32, kind="ExternalOutput").ap()

    with tile.TileContext(nc) as tc:
        with (
            tc.tile_pool(name="singles", bufs=1) as singles,
            tc.tile_pool(name="encp", bufs=3) as encp,
            tc.tile_pool(name="energy", bufs=4) as ep,
            tc.tile_pool(name="kpsum", bufs=4, space="PSUM") as kps,
            tc.tile_pool(name="apsum", bufs=2, space="PSUM") as aps,
            tc.tile_pool(name="spsum", bufs=1, space="PSUM") as sps,
            tc.tile_pool(name="stats", bufs=1) as stats,
        ):
            wk_sb = singles.tile([FT, HC, H], bf16)
            wq_sb = singles.tile([FT, HC, H], f32r)
            hsel_sb = singles.tile([FT, HC, ntp], f32r)
            v_sb = singles.tile([FT, HC], f32r)
            mask_sb = singles.tile([FT, ng, CW], f32)
            qsel_sb = singles.tile([FT, HC, ntp], f32)
            attn_sb = singles.tile([FT, ng, CW], f32)
            asn_sb = singles.tile([FT, nsub, NB], bf16)
            asn2_sb = singles.tile([NB, nsub, FT], bf16)
            dbias_sb = singles.tile([NB, 1], f32)

            for hc in range(HC):
                nc.sync.dma_start(out=wk_sb[:, hc, :],
                                  in_=wkT[hc * FT:(hc + 1) * FT, :])
                nc.sync.dma_start(out=wq_sb[:, hc, :],
                                  in_=wqT[hc * FT:(hc + 1) * FT, :])
                nc.sync.dma_start(out=hsel_sb[:, hc, :],
                                  in_=hselT[hc * FT:(hc + 1) * FT, :])
            nc.sync.dma_start(out=v_sb[:], in_=vp[:])
            nc.sync.dma_start(out=mask_sb[:], in_=maskP[:])
            nc.sync.dma_start(out=asn_sb[:], in_=asn[:])
            nc.sync.dma_start(out=asn2_sb[:], in_=asn2[:])
            nc.sync.dma_start(out=dbias_sb[:], in_=dbias[:])

            # q projection for every tile: qsel[f, t] = sum_h WqT[h, f] *
            # hsel[h, t].  All 8 fc regions share one PSUM tile (start only
            # on the global first matmul; nothing reads until the stop).
            qp = aps.tile([FT, HC, ntp], f32, tag="gp")
            for hc in range(HC):
                for fc in range(HC):
                    nc.tensor.matmul(
                        qp[:, fc, :],
                        lhsT=wq_sb[:, hc, fc * FT:(fc + 1) * FT],
                        rhs=hsel_sb[:, hc, :],
                        start=(hc == 0 and fc == 0),
                        stop=(hc == HC - 1 and fc == HC - 1),
                        skip_group_check=True)
            nc.vector.tensor_copy(out=qsel_sb[:], in_=qp[:])
            # rows/cols never written by the chunk tails park at -30 so the
            # group exp yields ~0 there (and stays finite over iterations)
            nc.vector.memset(attn_sb[:], -30.0)

            def load_et(c, off, cw):
                et = encp.tile([FT, HC, CW], bf16, tag="et", name="et")
                nc.sync.dma_start(out=et[:, :, :cw],
                                  in_=encP[:, :, off:off + cw])
                return et

            args = (nc, tc, mybir, f32, f32r, bf16, Tanh, Exp,
                    cs, ng, nsub, load_et, out, ep, kps, aps, sps, stats,
                    wk_sb, v_sb, mask_sb, qsel_sb, attn_sb,
                    asn_sb, asn2_sb, dbias_sb)
            if loop_r > 1:
                with tc.For_i(0, loop_r, 1):
                    _body(*args, pipelined=True)
            else:
                _body(*args, pipelined=False)

    nc.compile()
    return nc


def _body(nc, tc, mybir, f32, f32r, bf16, Tanh, Exp,
          cs, ng, nsub, load_et, out, ep, kps, aps, sps, stats,
          wk_sb, v_sb, mask_sb, qsel_sb, attn_sb,
          asn_sb, asn2_sb, dbias_sb, pipelined=False):
    """Emit the per-iteration chunk loop + softmax tail.

    The v-matvec for a finished energy tile is emitted one PE group late
    (after matmul hc==4 of the next keys group) so the tanh latency never
    stalls the in-order PE queue.  Groups 0..ng-2 finish their softmax
    prefix (mask add, exp, denominator fold) while later chunks are
    still streaming keys, so only group ng-1's short chain sits at the
    iteration boundary (overlapped into the next iteration's keys when
    pipelined).
    """
    HC_ = HC
    nch = len(cs)
    pending_v = []  # (ap_tile, energy_ap, fc, chunk)
    state = {"ap": None, "dn": None, "rowsum": None, "rowsum_bf": None}

    def chunk_tail(c, cw, pap):
        # scatter the finished [1,cw] attn row into its 32-aligned chunk
        # row, folding in the mask/shift bias.  For a ragged chunk the
        # row's cols past cw keep their parked value; their (g, sub)
        # one-hot columns are zero so they never fold into a denominator.
        g, p = c // 4, c % 4
        nc.vector.tensor_add(
            out=attn_sb[32 * p:32 * p + 1, g, :cw], in0=pap[:, :cw],
            in1=mask_sb[32 * p:32 * p + 1, g, :cw])
        if c == min(4 * g + 3, nch - 1) and not (pipelined and g == ng - 1):
            group_tail(g)

    def group_tail(g):
        if state["rowsum"] is None:
            state["rowsum"] = stats.tile([FT, nsub], f32, tag="rs",
                                         name="rowsum")
            state["rowsum_bf"] = stats.tile([FT, nsub], bf16, tag="rsb",
                                            name="rowsum_bf")
            state["dn"] = sps.tile([NB, 1], f32, tag="dn", name="dn")
        for s in range(TPC):
            nc.scalar.activation(
                out=attn_sb[:, g, s * TS:(s + 1) * TS],
                in_=attn_sb[:, g, s * TS:(s + 1) * TS], func=Exp,
                bias=0.0, scale=1.0,
                accum_out=state["rowsum"][:, 4 * g + s:4 * g + s + 1])
        with nc.allow_low_precision(reason="bf16 one-hot matmul operand"):
            nc.vector.tensor_copy(
                out=state["rowsum_bf"][:, 4 * g:4 * g + 4],
                in_=state["rowsum"][:, 4 * g:4 * g + 4])
        for s in range(TPC):
            k = 4 * g + s
            nc.tensor.matmul(
                state["dn"][:], lhsT=asn_sb[:, k, :],
                rhs=state["rowsum_bf"][:, k:k + 1],
                start=(k == 0), stop=(k == nsub - 1))

    def iter_tail():
        denom = stats.tile([NB, 1], f32, tag="den", name="denom")
        nc.vector.tensor_add(out=denom[:], in0=state["dn"][:],
                             in1=dbias_sb[:])
        recip = stats.tile([NB, 1], bf16, tag="rc", name="recip")
        with nc.allow_low_precision(reason="bf16 one-hot matmul operand"):
            nc.vector.reciprocal(out=recip[:], in_=denom[:])
        rs_ps = sps.tile([FT, nsub], f32, tag="rsp", name="rs_ps")
        for k in range(nsub):
            nc.tensor.matmul(rs_ps[:, k:k + 1], lhsT=asn2_sb[:, k, :],
                             rhs=recip[:], start=True, stop=True,
                             skip_group_check=True)
        rslot = stats.tile([FT, nsub], f32, tag="rsl", name="rslot")
        nc.vector.tensor_copy(out=rslot[:], in_=rs_ps[:])
        for g in range(ng):
            for s in range(TPC):
                k = 4 * g + s
                nc.vector.tensor_scalar_mul(
                    attn_sb[:, g, s * TS:(s + 1) * TS],
                    attn_sb[:, g, s * TS:(s + 1) * TS],
                    rslot[:, k:k + 1])
            nc.sync.dma_start(out=out[g], in_=attn_sb[0:FT:32, g, :])

    def flush(n):
        for _ in range(min(n, len(pending_v))):
            pap, pen, pfc, pc, pcw = pending_v.pop(0)
            nc.tensor.matmul(
                pap[:, :pcw], lhsT=v_sb[:, pfc:pfc + 1], rhs=pen,
                start=(pfc == 0), stop=(pfc == HC_ - 1))
            if pfc == HC_ - 1:
                chunk_tail(pc, pcw, pap)

    def keys_group(et, fc, cw):
        kp = kps.tile([FT, CW], f32, tag="kp")
        for hc in range(HC_):
            nc.tensor.matmul(
                kp[:, :cw],
                lhsT=wk_sb[:, hc, fc * FT:(fc + 1) * FT],
                rhs=et[:, hc, :cw],
                start=(hc == 0), stop=(hc == HC_ - 1))
            if hc in (4, 7):
                flush(2)
        return kp

    if pipelined:
        # previous iteration's last-group exp/denominator fold + its full
        # softmax tail run here, overlapped with this iteration's early
        # keys matmuls
        group_tail(ng - 1)
        iter_tail()
    off = 0
    for c, cw in enumerate(cs):
        et = load_et(c, off, cw)
        state["ap"] = aps.tile([1, CW], f32, tag="gp", name="ap_")
        ap_ = state["ap"]
        nts = cw // TS
        for fc in range(HC_):
            kp = keys_group(et, fc, cw)
            en = ep.tile([FT, CW], f32r, tag="en")
            for s in range(nts):
                nc.scalar.activation(
                    out=en[:, s * TS:(s + 1) * TS],
                    in_=kp[:, s * TS:(s + 1) * TS], func=Tanh,
                    bias=qsel_sb[:, fc, 4 * c + s:4 * c + s + 1], scale=1.0)
            pending_v.append((ap_, en[:, :cw], fc, c, cw))
        off += cw
    flush(len(pending_v))
    if pipelined:
        return

    # ---- single-shot tail: denominators -> scale factors -> output ----
    iter_tail()


def _plan(lengths):
    """Bin-pack whole batches across cores to equalize live-tile counts.

    Returns (assignment, maxtiles): assignment[c] = list of
    (global_batch, n_tiles); deterministic in `lengths`.
    """
    ntiles = [max(1, int(-(-int(l) // TS))) for l in lengths]
    order = sorted(range(B), key=lambda b: (-ntiles[b], b))
    sums = [0] * NCORES
    counts = [0] * NCORES
    assignment = [[] for _ in range(NCORES)]
    for b in order:
        cand = sorted(range(NCORES), key=lambda c: (sums[c], counts[c], c))
        placed = False
        for c in cand:
            if counts[c] < NB:
                assignment[c].append((b, ntiles[b]))
                sums[c] += ntiles[b]
                counts[c] += 1
                placed = True
                break
        assert placed, "more than NB batches needed on one core"
    return assignment, max(sums)


def _prepare_in_maps(hidden, encoder_outputs, lengths, Wq, Wk, v):
    hidden = np.asarray(hidden, dtype=np.float32)
    enc = np.asarray(encoder_outputs, dtype=np.float32)
    lengths = np.asarray(lengths).astype(np.int64)
    Wq = np.asarray(Wq, dtype=np.float32)
    Wk = np.asarray(Wk, dtype=np.float32)
    v = np.asarray(v, dtype=np.float32)

    assignment, maxtiles = _plan(lengths)
    cs = _chunks_of(maxtiles)
    nch = len(cs)
    ng = (nch + 3) // 4
    nsub = ng * TPC
    ntp = maxtiles + (maxtiles % 2)
    tot = sum(cs)
    _CACHE["maxtiles"] = maxtiles
    placement = []  # (core, tile_idx, global_batch, seq_tile, nvalid)

    WkT = np.ascontiguousarray(Wk.T).astype(ml_dtypes.bfloat16)  # (H, H)
    WqT = np.ascontiguousarray(Wq.T)                             # (H, H)
    encT = enc.transpose(0, 2, 1)                                # (B, H, S)
    vp = np.ascontiguousarray(v.reshape(HC, FT).T)               # (128, 8)

    in_maps = []
    for c in range(NCORES):
        encP = np.zeros((FT, HC, tot), dtype=ml_dtypes.bfloat16)
        hselT = np.zeros((H, ntp), dtype=np.float32)
        maskP = np.full((FT, ng, CW), np.float32(-1e30), dtype=np.float32)
        asn = np.zeros((FT, nsub, NB), dtype=np.float32)
        dbias = np.ones((NB, 1), dtype=np.float32)
        t = 0
        for j, (b, ntb) in enumerate(assignment[c]):
            L = int(lengths[b])
            dbias[j, 0] = 0.0
            for st in range(ntb):
                ch, sub = t // TPC, t % TPC
                g, p = ch // 4, ch % 4
                encP[:, :, t * TS:(t + 1) * TS] = encT[
                    b, :, st * TS:(st + 1) * TS].reshape(HC, FT, TS
                                                         ).transpose(1, 0, 2)
                hselT[:, t] = hidden[b]
                nvalid = min(TS, L - st * TS)
                maskP[32 * p, g, sub * TS:sub * TS + nvalid] = -SHIFT
                asn[32 * p, 4 * g + sub, j] = 1.0
                placement.append((c, t, b, st, nvalid))
                t += 1
        asn2 = np.ascontiguousarray(
            asn.transpose(2, 1, 0)).astype(ml_dtypes.bfloat16)  # (NB,nsub,FT)
        in_maps.append({
            "encP": encP,
            "hselT": np.ascontiguousarray(hselT),
            "wkT": WkT,
            "wqT": WqT,
            "vp": vp,
            "maskP": maskP,
            "asn": asn.astype(ml_dtypes.bfloat16),
            "asn2": asn2,
            "dbias": dbias,
        })
    _CACHE["placement"] = placement
    return in_maps


def _get_nc():
    maxtiles = _CACHE.get("maxtiles", 33)
    key = ("nc", maxtiles)
    if key not in _CACHE:
        _CACHE[key] = _build(maxtiles=maxtiles)
    return _CACHE[key]


def _scatter(results):
    outs = np.zeros((B, S), dtype=np.float32)
    for (c, t, b, st, nvalid) in _CACHE["placement"]:
        ch, sub = t // TPC, t % TPC
        g, p = ch // 4, ch % 4
        outs[b, st * TS:st * TS + nvalid] = results[c]["out"][
            g, p, sub * TS:sub * TS + nvalid]
    return outs


def _run(in_maps, trace=False, **kw):
    from concourse.bass_utils import run_bass_kernel_spmd
    nc = _get_nc()
    res = run_bass_kernel_spmd(nc, in_maps, core_ids=list(range(NCORES)),
                               trace=trace, **kw)
    return _scatter(res.results), res


def kernel(hidden, encoder_outputs, lengths, Wq, Wk, v):
    in_maps = _prepare_in_maps(hidden, encoder_outputs, lengths, Wq, Wk, v)
    outs, _ = _run(in_maps, trace=False)
    return outs


# revision 8
# speedup vs baseline: 1.2283x; 1.0553x over previous
"""Bahdanau-style additive attention on 8 TRN2 NeuronCores.

Reference computation (B=32, S=2048, H=1024):
    query  = hidden @ Wq.T                      # (B, H)
    keys   = enc @ Wk.T                         # (B, S, H)
    energy = tanh(query[:, None, :] + keys)     # (B, S, H)
    attn   = energy @ v                         # (B, S)
    out    = softmax(mask(attn, lengths))       # (B, S)

Seq positions >= lengths[b] are masked to -inf and contribute exactly 0
to the softmax, so their keys/energy never need to be computed.

v3 layout: the host packs the *live* seq range of every batch into
64-col tiles (TS=64) and bin-packs whole batches across the 8 cores by
tile count.  Each core's tile stream is processed in 512-col matmul
chunks (CW=512, 8 tiles each; the ragged tail chunk is rem*64 wide) --
the matmul moving dim stays wide (LDWEIGHTS amortized) while the
padding quantum is 64, cutting streamed columns ~21% vs 512-slot
packing.  A chunk may mix tiles of different batches: the query bias
is applied per 64-col subtile by DVE tensor-scalar adds (PSUM keys +
per-partition qsel scalar -> SBUF energy), then ONE ACT tanh per
(chunk, h-block) covers the whole chunk.  Softmax bookkeeping runs at
subtile granularity with one-hot folds, so the SPMD program is
identical on all cores; all per-tile metadata is data.

Softmax uses a fixed logit shift (baked into the host mask bias)
instead of a running max -- exact for softmax, no max-reduce chains.
Per-batch denominators are formed on the PE with tiny one-hot
assignment matmuls and broadcast back the same way.

Per-chunk dataflow (keys matmuls bf16, ~213ns per [128x128]@[128x512]):
    - enc chunk arrives pre-packed [128, HC, cw] (one DMA); keys tile
      [128f, cw] accumulates 8 h-chunk matmuls in PSUM.
    - DVE adds the per-tile query scalar per subtile into SBUF.
    - ACT applies tanh in place.
    - PE contracts energy with v into the chunk's attn row (deferred
      ~1 PE group to hide the DVE+tanh latency).
    - per finished group of 4 chunks: DVE adds mask/shift, ACT exps per
      subtile with per-row accumulate, PE folds the (row, subtile) sums
      into per-batch denominators.
    - iteration tail: reciprocal, broadcast back to (row, subtile)
      scale factors via one-hot matmuls, scale, DMA out.  Host scatters
      tile rows back to (batch, seq).
"""

import sys

if "/opt/trn_rl_repo" not in sys.path:
    sys.path.insert(0, "/opt/trn_rl_repo")

import ml_dtypes
import numpy as np

B, S, H = 32, 2048, 1024
NCORES = 8
FT = 128           # partition tile (feature / h chunk)
HC = H // FT       # h chunks
TS = 64            # seq packing tile
CW = 512           # matmul chunk width
TPC = CW // TS     # tiles per full chunk (8)
NB = 8             # max distinct batches per core
SHIFT = 30.0       # constant logit shift baked into the mask bias

_CACHE = {}


def _chunks_of(maxtiles):
    """Chunk widths for a core holding `maxtiles` tile slots."""
    nfull, rem = divmod(maxtiles, TPC)
    cs = [CW] * nfull
    if rem:
        cs.append(rem * TS)
    return cs


def _build(loop_r=1, maxtiles=63):
    import concourse.bass as bass  # noqa: F401
    import concourse.tile as tile
    from concourse import bacc, mybir

    cs = _chunks_of(maxtiles)
    nch = len(cs)
    ng = (nch + 3) // 4
    nsub = ng * TPC               # (group, subtile) column slots
    ntp = maxtiles + (maxtiles % 2)  # fp32r moving/dest free dim must be even
    tot = sum(cs)
    f32 = mybir.dt.float32
    f32r = mybir.dt.float32r
    bf16 = mybir.dt.bfloat16
    Tanh = mybir.ActivationFunctionType.Tanh
    Exp = mybir.ActivationFunctionType.Exp

    nc = bacc.Bacc("TRN2", target_bir_lowering=False, debug=False,
                   num_devices=NCORES)

    encP = nc.dram_tensor("encP", [FT, HC, tot], bf16,
                          kind="ExternalInput").ap()
    hselT = nc.dram_tensor("hselT", [H, ntp], f32r, kind="ExternalInput").ap()
    wkT = nc.dram_tensor("wkT", [H, H], bf16, kind="ExternalInput").ap()
    wqT = nc.dram_tensor("wqT", [H, H], f32r, kind="ExternalInput").ap()
    vp = nc.dram_tensor("vp", [FT, HC], f32r, kind="ExternalInput").ap()
    maskP = nc.dram_tensor("maskP", [FT, ng, CW], f32,
                           kind="ExternalInput").ap()
    asn = nc.dram_tensor("asn", [FT, nsub, NB], bf16,
                         kind="ExternalInput").ap()
    asn2 = nc.dram_tensor("asn2", [NB, nsub, FT], bf16,
                          kind="ExternalInput").ap()
    dbias = nc.dram_tensor("dbias", [NB, 1], f32, kind="ExternalInput").ap()
    out = nc.dram_tensor("out", [ng, 4, CW], f32, kind="ExternalOutput").ap()

    with tile.TileContext(nc) as tc:
        with (
            tc.tile_pool(name="singles", bufs=1) as singles,
            tc.tile_pool(name="encp", bufs=3) as encp,
            tc.tile_pool(name="energy", bufs=4) as ep,
            tc.tile_pool(name="kpsum", bufs=4, space="PSUM") as kps,
            tc.tile_pool(name="apsum", bufs=2, space="PSUM") as aps,
            tc.tile_pool(name="spsum", bufs=1, space="PSUM") as sps,
            tc.tile_pool(name="stats", bufs=1) as stats,
        ):
            wk_sb = singles.tile([FT, HC, H], bf16)
            wq_sb = singles.tile([FT, HC, H], f32r)
            hsel_sb = singles.tile([FT, HC, ntp], f32r)
            v_sb = singles.tile([FT, HC], f32r)
            mask_sb = singles.tile([FT, ng, CW], f32)
            qsel_sb = singles.tile([FT, HC, ntp], f32)
            attn_sb = singles.tile([FT, ng, CW], f32)
            asn_sb = singles.tile([FT, nsub, NB], bf16)
            asn2_sb = singles.tile([NB, nsub, FT], bf16)
            dbias_sb = singles.tile([NB, 1], f32)

            for hc in range(HC):
                nc.sync.dma_start(out=wk_sb[:, hc, :],
                                  in_=wkT[hc * FT:(hc + 1) * FT, :])
                nc.sync.dma_start(out=wq_sb[:, hc, :],
                                  in_=wqT[hc * FT:(hc + 1) * FT, :])
                nc.sync.dma_start(out=hsel_sb[:, hc, :],
                                  in_=hselT[hc * FT:(hc + 1) * FT, :])
            nc.sync.dma_start(out=v_sb[:], in_=vp[:])
            nc.sync.dma_start(out=mask_sb[:], in_=maskP[:])
            nc.sync.dma_start(out=asn_sb[:], in_=asn[:])
            nc.sync.dma_start(out=asn2_sb[:], in_=asn2[:])
            nc.sync.dma_start(out=dbias_sb[:], in_=dbias[:])

            # q projection for every tile: qsel[f, t] = sum_h WqT[h, f] *
            # hsel[h, t].  All 8 fc regions share one PSUM tile (start only
            # on the global first matmul; nothing reads until the stop).
            qp = aps.tile([FT, HC, ntp], f32, tag="gp")
            for hc in range(HC):
                for fc in range(HC):
                    nc.tensor.matmul(
                        qp[:, fc, :],
                        lhsT=wq_sb[:, hc, fc * FT:(fc + 1) * FT],
                        rhs=hsel_sb[:, hc, :],
                        start=(hc == 0 and fc == 0),
                        stop=(hc == HC - 1 and fc == HC - 1),
                        skip_group_check=True)
            nc.vector.tensor_copy(out=qsel_sb[:], in_=qp[:])
            # rows/cols never written by the chunk tails park at -30 so the
            # group exp yields ~0 there (and stays finite over iterations)
            nc.vector.memset(attn_sb[:], -30.0)

            def load_et(c, off, cw):
                et = encp.tile([FT, HC, CW], bf16, tag="et", name="et")
                nc.sync.dma_start(out=et[:, :, :cw],
                                  in_=encP[:, :, off:off + cw])
                return et

            args = (nc, tc, mybir, f32, f32r, bf16, Tanh, Exp,
                    cs, ng, nsub, load_et, out, ep, kps, aps, sps, stats,
                    wk_sb, v_sb, mask_sb, qsel_sb, attn_sb,
                    asn_sb, asn2_sb, dbias_sb)
            if loop_r > 1:
                with tc.For_i(0, loop_r, 1):
                    _body(*args, pipelined=True)
            else:
                _body(*args, pipelined=False)

    nc.compile()
    return nc


def _body(nc, tc, mybir, f32, f32r, bf16, Tanh, Exp,
          cs, ng, nsub, load_et, out, ep, kps, aps, sps, stats,
          wk_sb, v_sb, mask_sb, qsel_sb, attn_sb,
          asn_sb, asn2_sb, dbias_sb, pipelined=False):
    """Emit the per-iteration chunk loop + softmax tail.

    The v-matvec for a finished energy tile is emitted one PE group late
    (after matmul hc==4 of the next keys group) so the DVE-add + tanh
    latency never stalls the in-order PE queue.  Groups 0..ng-2 finish
    their softmax prefix (mask add, exp, denominator fold) while later
    chunks are still streaming keys, so only group ng-1's short chain
    sits at the iteration boundary (overlapped into the next iteration's
    keys when pipelined).
    """
    HC_ = HC
    nch = len(cs)
    pending_v = []  # (ap_tile, energy_ap, fc, chunk, cw)
    state = {"ap": None, "dn": None, "rowsum": None, "rowsum_bf": None}

    def chunk_tail(c, cw, pap):
        # scatter the finished [1,cw] attn row into its 32-aligned chunk
        # row, folding in the mask/shift bias.  For a ragged chunk the
        # row's cols past cw keep their parked value; their one-hot
        # columns are zero so they never fold into a denominator.
        g, p = c // 4, c % 4
        nc.vector.tensor_add(
            out=attn_sb[32 * p:32 * p + 1, g, :cw], in0=pap[:, :cw],
            in1=mask_sb[32 * p:32 * p + 1, g, :cw])
        if c == min(4 * g + 3, nch - 1) and not (pipelined and g == ng - 1):
            group_tail(g)

    def group_tail(g):
        if state["rowsum"] is None:
            state["rowsum"] = stats.tile([FT, nsub], f32, tag="rs",
                                         name="rowsum")
            state["rowsum_bf"] = stats.tile([FT, nsub], bf16, tag="rsb",
                                            name="rowsum_bf")
            state["dn"] = sps.tile([NB, 1], f32, tag="dn", name="dn")
        for s in range(TPC):
            nc.scalar.activation(
                out=attn_sb[:, g, s * TS:(s + 1) * TS],
                in_=attn_sb[:, g, s * TS:(s + 1) * TS], func=Exp,
                bias=0.0, scale=1.0,
                accum_out=state["rowsum"][:, TPC * g + s:TPC * g + s + 1])
        with nc.allow_low_precision(reason="bf16 one-hot matmul operand"):
            nc.vector.tensor_copy(
                out=state["rowsum_bf"][:, TPC * g:TPC * (g + 1)],
                in_=state["rowsum"][:, TPC * g:TPC * (g + 1)])
        for s in range(TPC):
            k = TPC * g + s
            nc.tensor.matmul(
                state["dn"][:], lhsT=asn_sb[:, k, :],
                rhs=state["rowsum_bf"][:, k:k + 1],
                start=(k == 0), stop=(k == nsub - 1))

    def iter_tail():
        denom = stats.tile([NB, 1], f32, tag="den", name="denom")
        nc.vector.tensor_add(out=denom[:], in0=state["dn"][:],
                             in1=dbias_sb[:])
        recip = stats.tile([NB, 1], bf16, tag="rc", name="recip")
        with nc.allow_low_precision(reason="bf16 one-hot matmul operand"):
            nc.vector.reciprocal(out=recip[:], in_=denom[:])
        rs_ps = sps.tile([FT, nsub], f32, tag="rsp", name="rs_ps")
        for k in range(nsub):
            nc.tensor.matmul(rs_ps[:, k:k + 1], lhsT=asn2_sb[:, k, :],
                             rhs=recip[:], start=True, stop=True,
                             skip_group_check=True)
        rslot = stats.tile([FT, nsub], f32, tag="rsl", name="rslot")
        nc.vector.tensor_copy(out=rslot[:], in_=rs_ps[:])
        for g in range(ng):
            for s in range(TPC):
                k = TPC * g + s
                nc.vector.tensor_scalar_mul(
                    attn_sb[:, g, s * TS:(s + 1) * TS],
                    attn_sb[:, g, s * TS:(s + 1) * TS],
                    rslot[:, k:k + 1])
            nc.sync.dma_start(out=out[g], in_=attn_sb[0:FT:32, g, :])

    def flush(n):
        for _ in range(min(n, len(pending_v))):
            pap, pen, pfc, pc, pcw = pending_v.pop(0)
            nc.tensor.matmul(
                pap[:, :pcw], lhsT=v_sb[:, pfc:pfc + 1], rhs=pen,
                start=(pfc == 0), stop=(pfc == HC_ - 1))
            if pfc == HC_ - 1:
                chunk_tail(pc, pcw, pap)

    def keys_group(et, fc, cw):
        kp = kps.tile([FT, CW], f32, tag="kp")
        for hc in range(HC_):
            nc.tensor.matmul(
                kp[:, :cw],
                lhsT=wk_sb[:, hc, fc * FT:(fc + 1) * FT],
                rhs=et[:, hc, :cw],
                start=(hc == 0), stop=(hc == HC_ - 1))
            if hc in (4, 7):
                flush(2)
        return kp

    if pipelined:
        # previous iteration's last-group exp/denominator fold + its full
        # softmax tail run here, overlapped with this iteration's early
        # keys matmuls
        group_tail(ng - 1)
        iter_tail()
    off = 0
    for c, cw in enumerate(cs):
        et = load_et(c, off, cw)
        state["ap"] = aps.tile([1, CW], f32, tag="gp", name="ap_")
        ap_ = state["ap"]
        nts = cw // TS
        for fc in range(HC_):
            kp = keys_group(et, fc, cw)
            en = ep.tile([FT, CW], f32r, tag="en")
            # fused q-bias add: one DVE op per (chunk, fc) with the
            # per-tile scalar broadcast along each 64-col subtile
            kp3 = kp[:, :cw].rearrange("p (t s) -> p t s", s=TS)
            en3 = en[:, :cw].rearrange("p (t s) -> p t s", s=TS)
            qb = qsel_sb[:, fc, TPC * c:TPC * c + nts].unsqueeze(
                -1).broadcast_to([FT, nts, TS])
            nc.vector.tensor_add(out=en3, in0=kp3, in1=qb)
            nc.scalar.activation(out=en[:, :cw], in_=en[:, :cw], func=Tanh,
                                 bias=0.0, scale=1.0)
            pending_v.append((ap_, en[:, :cw], fc, c, cw))
        off += cw
    flush(len(pending_v))
    if pipelined:
        return

    # ---- single-shot tail: denominators -> scale factors -> output ----
    iter_tail()


def _plan(lengths):
    """Bin-pack whole batches across cores to equalize live-tile counts.

    Returns (assignment, maxtiles): assignment[c] = list of
    (global_batch, n_tiles); deterministic in `lengths`.
    """
    ntiles = [max(1, int(-(-int(l) // TS))) for l in lengths]
    order = sorted(range(B), key=lambda b: (-ntiles[b], b))
    sums = [0] * NCORES
    counts = [0] * NCORES
    assignment = [[] for _ in range(NCORES)]
    for b in order:
        cand = sorted(range(NCORES), key=lambda c: (sums[c], counts[c], c))
        placed = False
        for c in cand:
            if counts[c] < NB:
                assignment[c].append((b, ntiles[b]))
                sums[c] += ntiles[b]
                counts[c] += 1
                placed = True
                break
        assert placed, "more than NB batches needed on one core"
    return assignment, max(sums)


def _prepare_in_maps(hidden, encoder_outputs, lengths, Wq, Wk, v):
    hidden = np.asarray(hidden, dtype=np.float32)
    enc = np.asarray(encoder_outputs, dtype=np.float32)
    lengths = np.asarray(lengths).astype(np.int64)
    Wq = np.asarray(Wq, dtype=np.float32)
    Wk = np.asarray(Wk, dtype=np.float32)
    v = np.asarray(v, dtype=np.float32)

    assignment, maxtiles = _plan(lengths)
    cs = _chunks_of(maxtiles)
    nch = len(cs)
    ng = (nch + 3) // 4
    nsub = ng * TPC
    ntp = maxtiles + (maxtiles % 2)
    tot = sum(cs)
    _CACHE["maxtiles"] = maxtiles
    placement = []  # (core, tile_idx, global_batch, seq_tile, nvalid)

    WkT = np.ascontiguousarray(Wk.T).astype(ml_dtypes.bfloat16)  # (H, H)
    WqT = np.ascontiguousarray(Wq.T)                             # (H, H)
    encT = enc.transpose(0, 2, 1)                                # (B, H, S)
    vp = np.ascontiguousarray(v.reshape(HC, FT).T)               # (128, 8)

    in_maps = []
    for c in range(NCORES):
        encP = np.zeros((FT, HC, tot), dtype=ml_dtypes.bfloat16)
        hselT = np.zeros((H, ntp), dtype=np.float32)
        maskP = np.full((FT, ng, CW), np.float32(-1e30), dtype=np.float32)
        asn = np.zeros((FT, nsub, NB), dtype=np.float32)
        dbias = np.ones((NB, 1), dtype=np.float32)
        t = 0
        for j, (b, ntb) in enumerate(assignment[c]):
            L = int(lengths[b])
            dbias[j, 0] = 0.0
            for st in range(ntb):
                ch, sub = t // TPC, t % TPC
                g, p = ch // 4, ch % 4
                encP[:, :, t * TS:(t + 1) * TS] = encT[
                    b, :, st * TS:(st + 1) * TS].reshape(HC, FT, TS
                                                         ).transpose(1, 0, 2)
                hselT[:, t] = hidden[b]
                nvalid = min(TS, L - st * TS)
                maskP[32 * p, g, sub * TS:sub * TS + nvalid] = -SHIFT
                asn[32 * p, TPC * g + sub, j] = 1.0
                placement.append((c, t, b, st, nvalid))
                t += 1
        asn2 = np.ascontiguousarray(
            asn.transpose(2, 1, 0)).astype(ml_dtypes.bfloat16)  # (NB,nsub,FT)
        in_maps.append({
            "encP": encP,
            "hselT": np.ascontiguousarray(hselT),
            "wkT": WkT,
            "wqT": WqT,
            "vp": vp,
            "maskP": maskP,
            "asn": asn.astype(ml_dtypes.bfloat16),
            "asn2": asn2,
            "dbias": dbias,
        })
    _CACHE["placement"] = placement
    return in_maps


def _get_nc():
    maxtiles = _CACHE.get("maxtiles", 63)
    key = ("nc", maxtiles)
    if key not in _CACHE:
        _CACHE[key] = _build(maxtiles=maxtiles)
    return _CACHE[key]


def _scatter(results):
    outs = np.zeros((B, S), dtype=np.float32)
    for (c, t, b, st, nvalid) in _CACHE["placement"]:
        ch, sub = t // TPC, t % TPC
        g, p = ch // 4, ch % 4
        outs[b, st * TS:st * TS + nvalid] = results[c]["out"][
            g, p, sub * TS:sub * TS + nvalid]
    return outs


def _run(in_maps, trace=False, **kw):
    from concourse.bass_utils import run_bass_kernel_spmd
    nc = _get_nc()
    res = run_bass_kernel_spmd(nc, in_maps, core_ids=list(range(NCORES)),
                               trace=trace, **kw)
    return _scatter(res.results), res


def kernel(hidden, encoder_outputs, lengths, Wq, Wk, v):
    in_maps = _prepare_in_maps(hidden, encoder_outputs, lengths, Wq, Wk, v)
    outs, _ = _run(in_maps, trace=False)
    return outs
